# revision 9
# baseline (speedup 1.0000x reference)
"""Multi-head attention (B=1, N=4096, NH=8, HS=64, DM=512) on 8 Trainium2
NeuronCores, head-parallel: core h computes head h end-to-end.

Per-core Bass/Tile kernel (head h):
  inputs:  xT      [512, 4096]  x transposed (shared across cores)
           wqkvT   [512, 192]   [Wq_h^T | Wk_h^T | Wv_h^T]
           wprojT  [64, 512]    Wproj[:, h*64:(h+1)*64]^T
  outputs: att     [4096, 4096] softmax probabilities for head h (normalized)
           proju   [512, 4096]  Wproj_h @ (expS^T @ v)  -- unnormalized out^T
           rowsum  [128, NB]    rowsum[p, ib] = sum_j exp(s[ib*128+p, j])

  pass 1 (att): S tiles [128 q-rows, 512 k-cols] on PE -> exp on ACT (with
    accum_out rowsums) -> normalize on DVE -> DMA att rows out.
  pass 2 (out): S^T tiles [128 k-rows, 512 q-cols] on PE -> exp on ACT ->
    PV matmuls accumulating expS^T-weighted v -> projection matmuls -> DMA.

Host: out[i, :] = sum_h proju_h[:, i] / rowsum_h[i] + bproj; att stacked.

Per-stage matmul dtypes are configurable: float32r (TF32-like ~11-bit
mantissa, 2 cyc/row) / bfloat16 (1 cyc/row, keeps the PE HAM clock-gate
warm) / float32 (exact, 4 cyc/row). fp32-family matmuls do not register
as PE activity in the HAM monitor, so optional tiny bf16 "warmer"
matmuls keep the clock at 2.4 GHz.
"""

import numpy as np

B, N, NH, HS = 1, 4096, 8, 64
DM = NH * HS  # 512
SCALE = HS ** (-0.5)

PRECISION = "f32r"  # "f32r" | "bf16" | "mixed" | "f32"

_COMPILED = {}

# stage dtype table: (qkv, s, pv, proj, warmers)
_CONFIGS = {
    "f32r": ("float32r", "float32r", "float32r", "float32r", False),
    "bf16": ("bfloat16", "bfloat16", "bfloat16", "bfloat16", False),
    "mixed": ("float32r", "float32r", "bfloat16", "float32r", False),
    "f32": ("float32", "float32", "float32", "float32", True),
    # bisect configs for the bf16@4096 crash
    "s16": ("float32r", "bfloat16", "bfloat16", "float32r", False),
    "qkv16": ("bfloat16", "float32r", "float32r", "float32r", False),
    # v2: all-bf16 with PE array packing (row-packed S/S^T, col-packed PV)
    "bf16p": ("bfloat16", "bfloat16", "bfloat16", "bfloat16", False),
}


def _np_dt(name):
    import ml_dtypes

    return {
        "float32r": np.float32,
        "float32": np.float32,
        "bfloat16": ml_dtypes.bfloat16,
    }[name]


def _gen_kernel_packed(n_ctx: int):
    """v2: all-bf16 with PE array packing.

    S/S^T are K=64 contractions -> 2x row tiling: tile T0 reads SBUF
    partitions 0-63 (qT/kT), tile T8 reads partitions 64-127 (qk2, a
    DMA-made copy of qT/kT), concurrent on disjoint row halves of the
    PE. PV has M=64 -> 2x col tiling: even/odd j-chunks accumulate into
    psum partitions 0-63 / 64-127; the projection matmul contracts K=128
    against row-duplicated Wproj^T, summing the halves for free.
    """
    import concourse.bacc as bacc
    import concourse.tile as tile
    from concourse import mybir

    f32 = mybir.dt.float32
    bf16 = mybir.dt.bfloat16

    NB = n_ctx // 128
    NJ = n_ctx // 512
    NJ2 = (NJ + 1) // 2
    NB2 = (NB + 1) // 2
    KC = DM // 128

    nc = bacc.Bacc(None, target_bir_lowering=False)

    d_xt = nc.dram_tensor("xT", [DM, n_ctx], bf16, kind="ExternalInput")
    d_wqkv = nc.dram_tensor("wqkvT", [DM, 3 * HS], bf16, kind="ExternalInput")
    d_wproj = nc.dram_tensor("wprojT", [HS, DM], bf16, kind="ExternalInput")
    d_att = nc.dram_tensor("att", [n_ctx, n_ctx], f32, kind="ExternalOutput")
    d_proju = nc.dram_tensor("proju", [DM, n_ctx], f32, kind="ExternalOutput")
    d_rowsum = nc.dram_tensor("rowsum", [128, NB], f32, kind="ExternalOutput")

    with tile.TileContext(nc) as tc:
        with (
            tc.tile_pool(name="persist", bufs=1) as persist,
            tc.tile_pool(name="work", bufs=3) as work,
            tc.tile_pool(name="small", bufs=4) as small,
            tc.tile_pool(name="estp", bufs=1) as estp,
            tc.tile_pool(name="psA", bufs=3, space="PSUM") as psA,
            tc.tile_pool(name="pvp", bufs=2, space="PSUM") as pvp,
        ):
            # ---- load inputs ----
            xt = persist.tile([128, KC, n_ctx], bf16)
            nc.sync.dma_start(xt[:], d_xt[:].rearrange("(c p) n -> p c n", p=128))
            wqkv = persist.tile([128, KC, 3 * HS], bf16)
            nc.sync.dma_start(
                wqkv[:], d_wqkv[:].rearrange("(c p) w -> p c w", p=128)
            )
            wproj2 = persist.tile([128, DM], bf16)  # Wproj_h^T duplicated rows
            nc.sync.dma_start(wproj2[:HS, :], d_wproj[:])
            nc.sync.dma_start(wproj2[HS:, :], d_wproj[:])

            # ---- qkv projections ----
            qT = persist.tile([HS, n_ctx], bf16, tag="qT")
            kT = persist.tile([HS, n_ctx], bf16, tag="kT")
            qk2 = persist.tile([128, 2, n_ctx], bf16, tag="qk2")  # rows 64-127
            vv = persist.tile([128, NB, HS], bf16, tag="v")
            for nb in range(NJ):
                for t, w0 in ((qT, 0), (kT, HS)):
                    ps = psA.tile([128, 2, 512], f32, tag="mm")
                    for c in range(KC):
                        nc.tensor.matmul(
                            ps[:HS, 0, :],
                            wqkv[:, c, w0 : w0 + HS],
                            xt[:, c, nb * 512 : (nb + 1) * 512],
                            start=(c == 0),
                            stop=(c == KC - 1),
                        )
                    nc.vector.tensor_copy(
                        t[:, nb * 512 : (nb + 1) * 512], ps[:HS, 0, :]
                    )
            for jb in range(NB):
                ps = psA.tile([128, 2, 512], f32, tag="mm")
                for c in range(KC):
                    nc.tensor.matmul(
                        ps[:, 0, :HS],
                        xt[:, c, jb * 128 : (jb + 1) * 128],
                        wqkv[:, c, 2 * HS : 3 * HS],
                        start=(c == 0),
                        stop=(c == KC - 1),
                    )
                nc.vector.tensor_copy(vv[:, jb, :], ps[:, 0, :HS])
            # duplicate qT/kT onto partitions 64-127 for the T8 row tile
            nc.sync.dma_start(qk2[HS:, 0, :], qT[:, :])
            nc.sync.dma_start(qk2[HS:, 1, :], kT[:, :])

            # ---- pass 1: S -> exp -> normalize -> att  (row-tiled 2x) ----
            rowsum_sb = persist.tile([128, NB], f32, tag="rowsum")
            for ib in range(NB):
                ibs = slice(ib * 128, (ib + 1) * 128)
                exps = work.tile([128, n_ctx], f32, tag="exps")
                rsp = small.tile([128, NJ2], f32, tag="rsp")
                for nb2 in range(NJ2):
                    g = min(2, NJ - nb2 * 2)
                    ps = psA.tile([128, 2, 512], f32, tag="mm")
                    nc.tensor.matmul(
                        ps[:, 0, :],
                        qT[:, ibs],
                        kT[:, nb2 * 1024 : nb2 * 1024 + 512],
                        start=True,
                        stop=True,
                        tile_position=(0, 0),
                    )
                    if g == 2:
                        nc.tensor.matmul(
                            ps[:, 1, :],
                            qk2[HS:, 0, ibs],
                            qk2[HS:, 1, nb2 * 1024 + 512 : nb2 * 1024 + 1024],
                            start=True,
                            stop=True,
                            tile_position=(64, 0),
                        )
                    nc.scalar.activation(
                        exps[:, nb2 * 1024 : nb2 * 1024 + g * 512],
                        ps[:, :g, :],
                        mybir.ActivationFunctionType.Exp,
                        scale=SCALE,
                        accum_out=rsp[:, nb2 : nb2 + 1],
                    )
                rs = small.tile([128, 1], f32, tag="rs")
                nc.vector.reduce_sum(rs[:], rsp[:], axis=mybir.AxisListType.X)
                nc.vector.tensor_copy(rowsum_sb[:, ib : ib + 1], rs[:])
                recip = small.tile([128, 1], f32, tag="recip")
                nc.vector.reciprocal(recip[:], rs[:])
                eng = nc.vector if ib % 2 == 0 else nc.gpsimd
                eng.tensor_scalar_mul(exps[:], exps[:], recip[:])
                nc.sync.dma_start(d_att[ibs, :], exps[:])
            nc.sync.dma_start(d_rowsum[:], rowsum_sb[:])

            # ---- pass 2: S^T -> exp -> PV -> projection ----
            for i2 in range(NJ):
                i2s = slice(i2 * 512, (i2 + 1) * 512)
                est = estp.tile([128, NB, 512], bf16, tag="est")
                for jb2 in range(NB2):
                    g = min(2, NB - jb2 * 2)
                    ps = psA.tile([128, 2, 512], f32, tag="mm")
                    nc.tensor.matmul(
                        ps[:, 0, :],
                        kT[:, jb2 * 256 : jb2 * 256 + 128],
                        qT[:, i2s],
                        start=True,
                        stop=True,
                        tile_position=(0, 0),
                    )
                    if g == 2:
                        nc.tensor.matmul(
                            ps[:, 1, :],
                            qk2[HS:, 1, jb2 * 256 + 128 : jb2 * 256 + 256],
                            qk2[HS:, 0, i2s],
                            start=True,
                            stop=True,
                            tile_position=(64, 0),
                        )
                    nc.scalar.activation(
                        est[:, jb2 * 2 : jb2 * 2 + g, :],
                        ps[:, :g, :],
                        mybir.ActivationFunctionType.Exp,
                        scale=SCALE,
                    )
                pv = pvp.tile([128, 512], f32, tag="pv")
                for jb2 in range(NB2):
                    g = min(2, NB - jb2 * 2)
                    nc.tensor.matmul(
                        pv[:HS, :],
                        vv[:, jb2 * 2, :],
                        est[:, jb2 * 2, :],
                        start=(jb2 == 0),
                        stop=(jb2 == NB2 - 1),
                        tile_position=(0, 0),
                        skip_group_check=True,
                    )
                    if g == 2:
                        nc.tensor.matmul(
                            pv[HS:, :],
                            vv[:, jb2 * 2 + 1, :],
                            est[:, jb2 * 2 + 1, :],
                            start=(jb2 == 0),
                            stop=(jb2 == NB2 - 1),
                            tile_position=(0, 64),
                            skip_group_check=True,
                        )
                outu = small.tile([128, 512], bf16, tag="outu")
                nc.vector.tensor_copy(outu[:], pv[:])
                for c in range(KC):
                    ps = psA.tile([128, 2, 512], f32, tag="mm")
                    nc.tensor.matmul(
                        ps[:, 0, :],
                        wproj2[:, c * 128 : (c + 1) * 128],
                        outu[:],
                        start=True,
                        stop=True,
                    )
                    pj = work.tile([128, 512], f32, tag="pj")
                    nc.vector.tensor_copy(pj[:], ps[:, 0, :])
                    nc.sync.dma_start(
                        d_proju[c * 128 : (c + 1) * 128, i2s], pj[:]
                    )

    nc.compile()
    return nc


def _gen_kernel(n_ctx: int, precision: str):
    import concourse.bacc as bacc
    import concourse.tile as tile
    from concourse import mybir

    if precision == "bf16p":
        return _gen_kernel_packed(n_ctx)

    f32 = mybir.dt.float32
    bf16 = mybir.dt.bfloat16
    cfg = _CONFIGS[precision]
    dt_qkv, dt_s, dt_pv, dt_proj = (getattr(mybir.dt, n) for n in cfg[:4])
    warmers = cfg[4]

    NB = n_ctx // 128   # query/key 128-row blocks
    NJ = n_ctx // 512   # 512-wide column blocks
    NJ2 = (NJ + 1) // 2
    NB2 = (NB + 1) // 2
    KC = DM // 128      # 4 contraction chunks for qkv projections

    nc = bacc.Bacc(None, target_bir_lowering=False)

    d_xt = nc.dram_tensor("xT", [DM, n_ctx], dt_qkv, kind="ExternalInput")
    d_wqkv = nc.dram_tensor("wqkvT", [DM, 3 * HS], dt_qkv, kind="ExternalInput")
    d_wproj = nc.dram_tensor("wprojT", [HS, DM], dt_proj, kind="ExternalInput")
    d_att = nc.dram_tensor("att", [n_ctx, n_ctx], f32, kind="ExternalOutput")
    d_proju = nc.dram_tensor("proju", [DM, n_ctx], f32, kind="ExternalOutput")
    d_rowsum = nc.dram_tensor("rowsum", [128, NB], f32, kind="ExternalOutput")

    with tile.TileContext(nc) as tc:
        with (
            tc.tile_pool(name="persist", bufs=1) as persist,
            tc.tile_pool(name="work", bufs=3) as work,
            tc.tile_pool(name="small", bufs=4) as small,
            tc.tile_pool(name="expst", bufs=3) as expst_pool,
            tc.tile_pool(name="psum", bufs=2, space="PSUM") as psum,
            tc.tile_pool(name="psum_pv", bufs=2, space="PSUM") as psum_pv,
        ):
            # ---- HAM warmers: tiny bf16 matmuls that count as PE activity
            # (fp32-mode matmuls don't), keeping the clock gate at 2.4 GHz.
            if warmers:
                wsrc = persist.tile([1, 64], bf16, tag="wsrc")
                nc.vector.memset(wsrc[:], 0.0)
                wps = psum_pv.tile([1, 64], f32, tag="warm")

                def warm():
                    nc.tensor.matmul(
                        wps[:], wsrc[:, :1], wsrc[:], start=True, stop=True
                    )
            else:

                def warm():
                    pass

            # ---- load inputs ----
            xt = persist.tile([128, KC, n_ctx], dt_qkv)  # xT[c*128+p, :]
            nc.sync.dma_start(
                xt[:], d_xt[:].rearrange("(c p) n -> p c n", p=128)
            )
            wqkv = persist.tile([128, KC, 3 * HS], dt_qkv)
            nc.sync.dma_start(
                wqkv[:], d_wqkv[:].rearrange("(c p) w -> p c w", p=128)
            )
            wproj = persist.tile([HS, DM], dt_proj)
            nc.sync.dma_start(wproj[:], d_wproj[:])

            # ---- qkv projections ----
            # qT/kT [64, n_ctx] = W^T.T @ xT ; v [128, jb, 64] plain
            qT = persist.tile([HS, n_ctx], dt_s, tag="qT")
            kT = persist.tile([HS, n_ctx], dt_s, tag="kT")
            vv = persist.tile([128, NB, HS], dt_pv, tag="v")
            for nb in range(NJ):
                for t, w0 in ((qT, 0), (kT, HS)):
                    ps = psum.tile([128, 2, 512], f32, tag="mm")
                    for c in range(KC):
                        nc.tensor.matmul(
                            ps[:HS, 0, :],
                            wqkv[:, c, w0 : w0 + HS],
                            xt[:, c, nb * 512 : (nb + 1) * 512],
                            start=(c == 0),
                            stop=(c == KC - 1),
                        )
                    warm()
                    nc.vector.tensor_copy(
                        t[:, nb * 512 : (nb + 1) * 512], ps[:HS, 0, :]
                    )
            for jb in range(NB):
                ps = psum.tile([128, 2, 512], f32, tag="mm")
                for c in range(KC):
                    nc.tensor.matmul(
                        ps[:, 0, :HS],
                        xt[:, c, jb * 128 : (jb + 1) * 128],
                        wqkv[:, c, 2 * HS : 3 * HS],
                        start=(c == 0),
                        stop=(c == KC - 1),
                    )
                if jb % 2 == 0:
                    warm()
                nc.vector.tensor_copy(vv[:, jb, :], ps[:, 0, :HS])

            # ---- pass 1: S -> exp -> normalize -> att ----
            # j-blocks in pairs: two matmuls fill the two banks of one psum
            # tile, one ACT op exps over both (N=1024 amortizes overhead).
            rowsum_sb = persist.tile([128, NB], f32, tag="rowsum")
            for ib in range(NB):
                exps = work.tile([128, n_ctx], f32, tag="exps")
                rsp = small.tile([128, NJ2], f32, tag="rsp")
                for nb2 in range(NJ2):
                    g = min(2, NJ - nb2 * 2)
                    ps = psum.tile([128, 2, 512], f32, tag="mm")
                    for u in range(g):
                        nb = nb2 * 2 + u
                        nc.tensor.matmul(
                            ps[:, u, :],
                            qT[:, ib * 128 : (ib + 1) * 128],
                            kT[:, nb * 512 : (nb + 1) * 512],
                            start=True,
                            stop=True,
                        )
                    warm()
                    nc.scalar.activation(
                        exps[:, nb2 * 1024 : nb2 * 1024 + g * 512],
                        ps[:, :g, :],
                        mybir.ActivationFunctionType.Exp,
                        scale=SCALE,
                        accum_out=rsp[:, nb2 : nb2 + 1],
                    )
                rs = small.tile([128, 1], f32, tag="rs")
                nc.vector.reduce_sum(rs[:], rsp[:], axis=mybir.AxisListType.X)
                nc.vector.tensor_copy(rowsum_sb[:, ib : ib + 1], rs[:])
                recip = small.tile([128, 1], f32, tag="recip")
                nc.vector.reciprocal(recip[:], rs[:])
                nc.vector.tensor_scalar_mul(exps[:], exps[:], recip[:])
                nc.sync.dma_start(d_att[ib * 128 : (ib + 1) * 128, :], exps[:])
            nc.sync.dma_start(d_rowsum[:], rowsum_sb[:])

            # ---- pass 2: S^T -> exp -> PV -> projection ----
            for i2 in range(NJ):
                pv = psum_pv.tile([HS, 512], f32, tag="pv")
                for jb2 in range(NB2):
                    g = min(2, NB - jb2 * 2)
                    ps = psum.tile([128, 2, 512], f32, tag="mm")
                    for u in range(g):
                        jb = jb2 * 2 + u
                        nc.tensor.matmul(
                            ps[:, u, :],
                            kT[:, jb * 128 : (jb + 1) * 128],
                            qT[:, i2 * 512 : (i2 + 1) * 512],
                            start=True,
                            stop=True,
                        )
                    warm()
                    est = expst_pool.tile([128, 2, 512], dt_pv, tag="expst")
                    nc.scalar.activation(
                        est[:, :g, :],
                        ps[:, :g, :],
                        mybir.ActivationFunctionType.Exp,
                        scale=SCALE,
                    )
                    for u in range(g):
                        jb = jb2 * 2 + u
                        nc.tensor.matmul(
                            pv[:],
                            vv[:, jb, :],
                            est[:, u, :],
                            start=(jb == 0),
                            stop=(jb == NB - 1),
                        )
                outu = small.tile([HS, 512], dt_proj, tag="outu")
                nc.vector.tensor_copy(outu[:], pv[:])
                for c in range(KC):
                    ps = psum.tile([128, 2, 512], f32, tag="mm")
                    nc.tensor.matmul(
                        ps[:, 0, :],
                        wproj[:, c * 128 : (c + 1) * 128],
                        outu[:],
                        start=True,
                        stop=True,
                    )
                    pj = work.tile([128, 512], f32, tag="pj")
                    nc.vector.tensor_copy(pj[:], ps[:, 0, :])
                    nc.sync.dma_start(
                        d_proju[
                            c * 128 : (c + 1) * 128, i2 * 512 : (i2 + 1) * 512
                        ],
                        pj[:],
                    )

    nc.compile()
    return nc


def _get_kernel(n_ctx: int, precision: str):
    key = (n_ctx, precision)
    if key not in _COMPILED:
        _COMPILED[key] = _gen_kernel(n_ctx, precision)
    return _COMPILED[key]


def run_cores(x, Wqkv, Wproj, n_ctx=N, precision=PRECISION, trace=False):
    """Run the 8-core kernel; returns (list of per-core result dicts, perf)."""
    from concourse.bass_utils import run_bass_kernel_spmd

    nc = _get_kernel(n_ctx, precision)
    cfg = _CONFIGS[precision]
    np_qkv, np_proj = _np_dt(cfg[0]), _np_dt(cfg[3])

    xT = np.ascontiguousarray(x.reshape(n_ctx, DM).T).astype(np_qkv)
    in_maps = []
    for h in range(NH):
        sl = slice(h * HS, (h + 1) * HS)
        wq = Wqkv[0 * DM + h * HS : 0 * DM + (h + 1) * HS]  # [64, 512]
        wk = Wqkv[1 * DM + h * HS : 1 * DM + (h + 1) * HS]
        wv = Wqkv[2 * DM + h * HS : 2 * DM + (h + 1) * HS]
        wqkvT = np.ascontiguousarray(
            np.concatenate([wq, wk, wv], axis=0).T
        ).astype(np_qkv)  # [512, 192]
        wprojT = np.ascontiguousarray(Wproj[:, sl].T).astype(np_proj)
        in_maps.append({"xT": xT, "wqkvT": wqkvT, "wprojT": wprojT})

    res = run_bass_kernel_spmd(
        nc, in_maps, core_ids=list(range(NH)), trace=trace
    )
    return res.results, res


def kernel(x, Wqkv, Wproj, bproj):
    x = np.asarray(x, dtype=np.float32)
    Wqkv = np.asarray(Wqkv, dtype=np.float32)
    Wproj = np.asarray(Wproj, dtype=np.float32)
    bproj = np.asarray(bproj, dtype=np.float32)

    results, _ = run_cores(x, Wqkv, Wproj)

    att = np.empty((B, NH, N, N), dtype=np.float32)
    out = np.zeros((N, DM), dtype=np.float32)
    for h in range(NH):
        r = results[h]
        att[0, h] = r["att"]
        rs = r["rowsum"].T.reshape(N).astype(np.float64)  # rowsum for query i
        out += (r["proju"] / rs[None, :]).T.astype(np.float32)
    out += bproj[None, :]
    return out.reshape(B, N, DM).astype(np.float32), att


# revision 10
# speedup vs baseline: 2.7709x; 2.7709x over previous
"""Multi-head attention (B=1, N=4096, NH=8, HS=64, DM=512) on 8 Trainium2
NeuronCores, head-parallel: core h computes head h end-to-end.

Per-core Bass/Tile kernel (head h):
  inputs:  xT      [512, 4096]  x transposed (shared across cores)
           wqkvT   [512, 192]   [Wq_h^T | Wk_h^T | Wv_h^T]
           wprojT  [64, 512]    Wproj[:, h*64:(h+1)*64]^T
  outputs: att     [4096, 4096] softmax probabilities for head h (normalized)
           proju   [512, 4096]  Wproj_h @ (expS^T @ v)  -- unnormalized out^T
           rowsum  [128, NB]    rowsum[p, ib] = sum_j exp(s[ib*128+p, j])

  pass 1 (att): S tiles [128 q-rows, 512 k-cols] on PE -> exp on ACT (with
    accum_out rowsums) -> normalize on DVE -> DMA att rows out.
  pass 2 (out): S^T tiles [128 k-rows, 512 q-cols] on PE -> exp on ACT ->
    PV matmuls accumulating expS^T-weighted v -> projection matmuls -> DMA.

Host: out[i, :] = sum_h proju_h[:, i] / rowsum_h[i] + bproj; att stacked.

Per-stage matmul dtypes are configurable: float32r (TF32-like ~11-bit
mantissa, 2 cyc/row) / bfloat16 (1 cyc/row, keeps the PE HAM clock-gate
warm) / float32 (exact, 4 cyc/row). fp32-family matmuls do not register
as PE activity in the HAM monitor, so optional tiny bf16 "warmer"
matmuls keep the clock at 2.4 GHz.
"""

import numpy as np

B, N, NH, HS = 1, 4096, 8, 64
DM = NH * HS  # 512
SCALE = HS ** (-0.5)

PRECISION = "f32r"  # "f32r" | "bf16" | "mixed" | "f32"

_COMPILED = {}

# stage dtype table: (qkv, s, pv, proj, warmers)
_CONFIGS = {
    "f32r": ("float32r", "float32r", "float32r", "float32r", False),
    "bf16": ("bfloat16", "bfloat16", "bfloat16", "bfloat16", False),
    "mixed": ("float32r", "float32r", "bfloat16", "float32r", False),
    "f32": ("float32", "float32", "float32", "float32", True),
    # bisect configs for the bf16@4096 crash
    "s16": ("float32r", "bfloat16", "bfloat16", "float32r", False),
    "qkv16": ("bfloat16", "float32r", "float32r", "float32r", False),
    # v2: all-bf16 with PE array packing (row-packed S/S^T, col-packed PV)
    "bf16p": ("bfloat16", "bfloat16", "bfloat16", "bfloat16", False),
}


def _np_dt(name):
    import ml_dtypes

    return {
        "float32r": np.float32,
        "float32": np.float32,
        "bfloat16": ml_dtypes.bfloat16,
    }[name]


def _gen_kernel_packed(n_ctx: int):
    """v3: all-bf16 with PE array packing.

    S/S^T are K=64 contractions -> 2x row tiling: tile T0 reads SBUF
    partitions 0-63 (qT/kT), tile T8 reads partitions 64-127 (qk2, a
    DMA-made copy of qT/kT), concurrent on disjoint row halves of the
    PE. PV has M=64 -> 2x col tiling: even/odd j-chunks accumulate into
    psum partitions 0-63 / 64-127; the projection matmul contracts K=128
    against row-duplicated Wproj^T, summing the halves for free.
    QKV q/k are col-packed (q -> psum 0-63, k -> psum 64-127, shared x
    stream). Pass 2 is software-pipelined: PV/proj of block i2-1 are
    emitted between the ACT-gated S^T pairs of block i2 to fill PE idle.
    """
    import concourse.bacc as bacc
    import concourse.tile as tile
    from concourse import mybir

    f32 = mybir.dt.float32
    bf16 = mybir.dt.bfloat16

    NB = n_ctx // 128
    NJ = n_ctx // 512
    NB2 = (NB + 1) // 2
    KC = DM // 128
    NJ4 = (NJ + 3) // 4  # pass-1 groups of 4 j-blocks (one N=2048 ACT op)

    nc = bacc.Bacc(None, target_bir_lowering=False)

    d_xt = nc.dram_tensor("xT", [DM, n_ctx], bf16, kind="ExternalInput")
    d_wqkv = nc.dram_tensor("wqkvT", [DM, 3 * HS], bf16, kind="ExternalInput")
    d_wproj = nc.dram_tensor("wprojT", [HS, DM], bf16, kind="ExternalInput")
    d_att = nc.dram_tensor("att", [n_ctx, n_ctx], f32, kind="ExternalOutput")
    d_proju = nc.dram_tensor("proju", [DM, n_ctx], f32, kind="ExternalOutput")
    d_rowsum = nc.dram_tensor("rowsum", [128, NB], f32, kind="ExternalOutput")

    with tile.TileContext(nc) as tc:
        with (
            tc.tile_pool(name="persist", bufs=1) as persist,
            tc.tile_pool(name="work", bufs=2) as work,
            tc.tile_pool(name="small", bufs=4) as small,
            tc.tile_pool(name="estp", bufs=2) as estp,
        ):
            # ---- load inputs ----
            xt = persist.tile([128, KC, n_ctx], bf16)
            nc.sync.dma_start(xt[:], d_xt[:].rearrange("(c p) n -> p c n", p=128))
            wqkv = persist.tile([128, KC, 3 * HS], bf16)
            nc.sync.dma_start(
                wqkv[:], d_wqkv[:].rearrange("(c p) w -> p c w", p=128)
            )
            wproj2 = persist.tile([128, DM], bf16)  # Wproj_h^T duplicated rows
            nc.sync.dma_start(wproj2[:HS, :], d_wproj[:])
            nc.sync.dma_start(wproj2[HS:, :], d_wproj[:])

            qT = persist.tile([HS, n_ctx], bf16, tag="qT")
            kT = persist.tile([HS, n_ctx], bf16, tag="kT")
            qk2 = persist.tile([128, 2, n_ctx], bf16, tag="qk2")  # rows 64-127
            vv = persist.tile([128, NB, HS], bf16, tag="v")
            rowsum_sb = persist.tile([128, NB], f32, tag="rowsum")

            # ======== scope 1: qkv + pass 1 (8 psum banks) ========
            with tc.tile_pool(name="psA", bufs=2, space="PSUM") as psA:
                # q/k col-packed: T0 -> q (psum 0-63), T1 -> k (psum 64-127)
                for nb in range(NJ):
                    ps = psA.tile([128, 4, 512], f32, tag="mm")
                    for c in range(KC):
                        nc.tensor.matmul(
                            ps[:HS, 0, :],
                            wqkv[:, c, 0:HS],
                            xt[:, c, nb * 512 : (nb + 1) * 512],
                            start=(c == 0),
                            stop=(c == KC - 1),
                            tile_position=(0, 0),
                        )
                        nc.tensor.matmul(
                            ps[HS:, 0, :],
                            wqkv[:, c, HS : 2 * HS],
                            xt[:, c, nb * 512 : (nb + 1) * 512],
                            start=(c == 0),
                            stop=(c == KC - 1),
                            tile_position=(0, 64),
                        )
                    nc.vector.tensor_copy(
                        qT[:, nb * 512 : (nb + 1) * 512], ps[:HS, 0, :]
                    )
                    nc.vector.tensor_copy(
                        qk2[HS:, 1, nb * 512 : (nb + 1) * 512], ps[HS:, 0, :]
                    )
                for jb in range(NB):
                    ps = psA.tile([128, 4, 512], f32, tag="mm")
                    for c in range(KC):
                        nc.tensor.matmul(
                            ps[:, 0, :HS],
                            xt[:, c, jb * 128 : (jb + 1) * 128],
                            wqkv[:, c, 2 * HS : 3 * HS],
                            start=(c == 0),
                            stop=(c == KC - 1),
                        )
                    nc.vector.tensor_copy(vv[:, jb, :], ps[:, 0, :HS])
                # distribute q/k copies for the second row half
                nc.sync.dma_start(kT[:, :], qk2[HS:, 1, :])
                nc.sync.dma_start(qk2[HS:, 0, :], qT[:, :])

                # ---- pass 1: S -> exp(accum) -> normalize -> att ----
                for ib in range(NB):
                    ibs = slice(ib * 128, (ib + 1) * 128)
                    exps = work.tile([128, n_ctx], f32, tag="exps")
                    rsp = small.tile([128, NJ4], f32, tag="rsp")
                    for nb4 in range(NJ4):
                        g = min(4, NJ - nb4 * 4)  # j-blocks in this group
                        ps = psA.tile([128, 4, 512], f32, tag="mm")
                        for u in range(0, g, 2):
                            nb = nb4 * 4 + u
                            nc.tensor.matmul(
                                ps[:, u, :],
                                qT[:, ibs],
                                kT[:, nb * 512 : (nb + 1) * 512],
                                start=True,
                                stop=True,
                                tile_position=(0, 0),
                            )
                            if u + 1 < g:
                                nc.tensor.matmul(
                                    ps[:, u + 1, :],
                                    qk2[HS:, 0, ibs],
                                    qk2[
                                        HS:,
                                        1,
                                        (nb + 1) * 512 : (nb + 2) * 512,
                                    ],
                                    start=True,
                                    stop=True,
                                    tile_position=(64, 0),
                                )
                        nc.scalar.activation(
                            exps[:, nb4 * 2048 : nb4 * 2048 + g * 512],
                            ps[:, :g, :],
                            mybir.ActivationFunctionType.Exp,
                            scale=SCALE,
                            accum_out=rsp[:, nb4 : nb4 + 1],
                        )
                    rs = small.tile([128, 1], f32, tag="rs")
                    nc.vector.reduce_sum(
                        rs[:], rsp[:], axis=mybir.AxisListType.X
                    )
                    nc.vector.tensor_copy(rowsum_sb[:, ib : ib + 1], rs[:])
                    recip = small.tile([128, 1], f32, tag="recip")
                    nc.vector.reciprocal(recip[:], rs[:])
                    nc.vector.tensor_scalar_mul(exps[:], exps[:], recip[:])
                    nc.sync.dma_start(d_att[ibs, :], exps[:])
                nc.sync.dma_start(d_rowsum[:], rowsum_sb[:])

            # ======== scope 2: pass 2 (6 + 2 psum banks) ========
            with (
                tc.tile_pool(name="psB", bufs=3, space="PSUM") as psB,
                tc.tile_pool(name="pvp", bufs=2, space="PSUM") as pvp,
            ):

                def st_pair(est, i2, jb2):
                    """row-packed S^T pair (jb = 2*jb2, 2*jb2+1) + exp."""
                    i2s = slice(i2 * 512, (i2 + 1) * 512)
                    g = min(2, NB - jb2 * 2)
                    ps = psB.tile([128, 2, 512], f32, tag="mm")
                    nc.tensor.matmul(
                        ps[:, 0, :],
                        kT[:, jb2 * 256 : jb2 * 256 + 128],
                        qT[:, i2s],
                        start=True,
                        stop=True,
                        tile_position=(0, 0),
                    )
                    if g == 2:
                        nc.tensor.matmul(
                            ps[:, 1, :],
                            qk2[HS:, 1, jb2 * 256 + 128 : jb2 * 256 + 256],
                            qk2[HS:, 0, i2s],
                            start=True,
                            stop=True,
                            tile_position=(64, 0),
                        )
                    nc.scalar.activation(
                        est[:, jb2 * 2 : jb2 * 2 + g, :],
                        ps[:, :g, :],
                        mybir.ActivationFunctionType.Exp,
                        scale=SCALE,
                    )

                def pv_pair(est, pv, jb2):
                    """col-packed PV pair accumulating into pv halves."""
                    g = min(2, NB - jb2 * 2)
                    nc.tensor.matmul(
                        pv[:HS, :],
                        vv[:, jb2 * 2, :],
                        est[:, jb2 * 2, :],
                        start=(jb2 == 0),
                        stop=(jb2 == NB2 - 1),
                        tile_position=(0, 0),
                        skip_group_check=True,
                    )
                    if g == 2:
                        nc.tensor.matmul(
                            pv[HS:, :],
                            vv[:, jb2 * 2 + 1, :],
                            est[:, jb2 * 2 + 1, :],
                            start=(jb2 == 0),
                            stop=(jb2 == NB2 - 1),
                            tile_position=(0, 64),
                            skip_group_check=True,
                        )

                def proj_out(pv, i2):
                    i2s = slice(i2 * 512, (i2 + 1) * 512)
                    outu = small.tile([128, 512], bf16, tag="outu")
                    nc.vector.tensor_copy(outu[:], pv[:])
                    for c in range(KC):
                        ps = psB.tile([128, 2, 512], f32, tag="mm")
                        nc.tensor.matmul(
                            ps[:, 0, :],
                            wproj2[:, c * 128 : (c + 1) * 128],
                            outu[:],
                            start=True,
                            stop=True,
                        )
                        pj = work.tile([128, 512], f32, tag="pj")
                        nc.vector.tensor_copy(pj[:], ps[:, 0, :])
                        nc.sync.dma_start(
                            d_proju[c * 128 : (c + 1) * 128, i2s], pj[:]
                        )

                # software pipeline over i2: S^T/exp of i2 interleaved (in
                # batches of 2 pairs) with PV of i2-1; proj of i2-1 at end.
                ests = {}
                pvs = {}
                ests[0] = estp.tile([128, NB, 512], bf16, tag="est", name="est0")
                for jb2 in range(NB2):
                    st_pair(ests[0], 0, jb2)
                for i2 in range(1, NJ + 1):
                    pvs[i2 - 1] = pvp.tile([128, 512], f32, tag="pv",
                                           name=f"pv{i2 - 1}")
                    if i2 < NJ:
                        ests[i2] = estp.tile([128, NB, 512], bf16, tag="est",
                                             name=f"est{i2}")
                    for jb2 in range(0, NB2, 2):
                        if i2 < NJ:
                            for k in range(jb2, min(jb2 + 2, NB2)):
                                st_pair(ests[i2], i2, k)
                        for k in range(jb2, min(jb2 + 2, NB2)):
                            pv_pair(ests[i2 - 1], pvs[i2 - 1], k)
                    proj_out(pvs[i2 - 1], i2 - 1)

    nc.compile()
    return nc


def _gen_kernel(n_ctx: int, precision: str):
    import concourse.bacc as bacc
    import concourse.tile as tile
    from concourse import mybir

    if precision == "bf16p":
        return _gen_kernel_packed(n_ctx)

    f32 = mybir.dt.float32
    bf16 = mybir.dt.bfloat16
    cfg = _CONFIGS[precision]
    dt_qkv, dt_s, dt_pv, dt_proj = (getattr(mybir.dt, n) for n in cfg[:4])
    warmers = cfg[4]

    NB = n_ctx // 128   # query/key 128-row blocks
    NJ = n_ctx // 512   # 512-wide column blocks
    NJ2 = (NJ + 1) // 2
    NB2 = (NB + 1) // 2
    KC = DM // 128      # 4 contraction chunks for qkv projections

    nc = bacc.Bacc(None, target_bir_lowering=False)

    d_xt = nc.dram_tensor("xT", [DM, n_ctx], dt_qkv, kind="ExternalInput")
    d_wqkv = nc.dram_tensor("wqkvT", [DM, 3 * HS], dt_qkv, kind="ExternalInput")
    d_wproj = nc.dram_tensor("wprojT", [HS, DM], dt_proj, kind="ExternalInput")
    d_att = nc.dram_tensor("att", [n_ctx, n_ctx], f32, kind="ExternalOutput")
    d_proju = nc.dram_tensor("proju", [DM, n_ctx], f32, kind="ExternalOutput")
    d_rowsum = nc.dram_tensor("rowsum", [128, NB], f32, kind="ExternalOutput")

    with tile.TileContext(nc) as tc:
        with (
            tc.tile_pool(name="persist", bufs=1) as persist,
            tc.tile_pool(name="work", bufs=3) as work,
            tc.tile_pool(name="small", bufs=4) as small,
            tc.tile_pool(name="expst", bufs=3) as expst_pool,
            tc.tile_pool(name="psum", bufs=2, space="PSUM") as psum,
            tc.tile_pool(name="psum_pv", bufs=2, space="PSUM") as psum_pv,
        ):
            # ---- HAM warmers: tiny bf16 matmuls that count as PE activity
            # (fp32-mode matmuls don't), keeping the clock gate at 2.4 GHz.
            if warmers:
                wsrc = persist.tile([1, 64], bf16, tag="wsrc")
                nc.vector.memset(wsrc[:], 0.0)
                wps = psum_pv.tile([1, 64], f32, tag="warm")

                def warm():
                    nc.tensor.matmul(
                        wps[:], wsrc[:, :1], wsrc[:], start=True, stop=True
                    )
            else:

                def warm():
                    pass

            # ---- load inputs ----
            xt = persist.tile([128, KC, n_ctx], dt_qkv)  # xT[c*128+p, :]
            nc.sync.dma_start(
                xt[:], d_xt[:].rearrange("(c p) n -> p c n", p=128)
            )
            wqkv = persist.tile([128, KC, 3 * HS], dt_qkv)
            nc.sync.dma_start(
                wqkv[:], d_wqkv[:].rearrange("(c p) w -> p c w", p=128)
            )
            wproj = persist.tile([HS, DM], dt_proj)
            nc.sync.dma_start(wproj[:], d_wproj[:])

            # ---- qkv projections ----
            # qT/kT [64, n_ctx] = W^T.T @ xT ; v [128, jb, 64] plain
            qT = persist.tile([HS, n_ctx], dt_s, tag="qT")
            kT = persist.tile([HS, n_ctx], dt_s, tag="kT")
            vv = persist.tile([128, NB, HS], dt_pv, tag="v")
            for nb in range(NJ):
                for t, w0 in ((qT, 0), (kT, HS)):
                    ps = psum.tile([128, 2, 512], f32, tag="mm")
                    for c in range(KC):
                        nc.tensor.matmul(
                            ps[:HS, 0, :],
                            wqkv[:, c, w0 : w0 + HS],
                            xt[:, c, nb * 512 : (nb + 1) * 512],
                            start=(c == 0),
                            stop=(c == KC - 1),
                        )
                    warm()
                    nc.vector.tensor_copy(
                        t[:, nb * 512 : (nb + 1) * 512], ps[:HS, 0, :]
                    )
            for jb in range(NB):
                ps = psum.tile([128, 2, 512], f32, tag="mm")
                for c in range(KC):
                    nc.tensor.matmul(
                        ps[:, 0, :HS],
                        xt[:, c, jb * 128 : (jb + 1) * 128],
                        wqkv[:, c, 2 * HS : 3 * HS],
                        start=(c == 0),
                        stop=(c == KC - 1),
                    )
                if jb % 2 == 0:
                    warm()
                nc.vector.tensor_copy(vv[:, jb, :], ps[:, 0, :HS])

            # ---- pass 1: S -> exp -> normalize -> att ----
            # j-blocks in pairs: two matmuls fill the two banks of one psum
            # tile, one ACT op exps over both (N=1024 amortizes overhead).
            rowsum_sb = persist.tile([128, NB], f32, tag="rowsum")
            for ib in range(NB):
                exps = work.tile([128, n_ctx], f32, tag="exps")
                rsp = small.tile([128, NJ2], f32, tag="rsp")
                for nb2 in range(NJ2):
                    g = min(2, NJ - nb2 * 2)
                    ps = psum.tile([128, 2, 512], f32, tag="mm")
                    for u in range(g):
                        nb = nb2 * 2 + u
                        nc.tensor.matmul(
                            ps[:, u, :],
                            qT[:, ib * 128 : (ib + 1) * 128],
                            kT[:, nb * 512 : (nb + 1) * 512],
                            start=True,
                            stop=True,
                        )
                    warm()
                    nc.scalar.activation(
                        exps[:, nb2 * 1024 : nb2 * 1024 + g * 512],
                        ps[:, :g, :],
                        mybir.ActivationFunctionType.Exp,
                        scale=SCALE,
                        accum_out=rsp[:, nb2 : nb2 + 1],
                    )
                rs = small.tile([128, 1], f32, tag="rs")
                nc.vector.reduce_sum(rs[:], rsp[:], axis=mybir.AxisListType.X)
                nc.vector.tensor_copy(rowsum_sb[:, ib : ib + 1], rs[:])
                recip = small.tile([128, 1], f32, tag="recip")
                nc.vector.reciprocal(recip[:], rs[:])
                nc.vector.tensor_scalar_mul(exps[:], exps[:], recip[:])
                nc.sync.dma_start(d_att[ib * 128 : (ib + 1) * 128, :], exps[:])
            nc.sync.dma_start(d_rowsum[:], rowsum_sb[:])

            # ---- pass 2: S^T -> exp -> PV -> projection ----
            for i2 in range(NJ):
                pv = psum_pv.tile([HS, 512], f32, tag="pv")
                for jb2 in range(NB2):
                    g = min(2, NB - jb2 * 2)
                    ps = psum.tile([128, 2, 512], f32, tag="mm")
                    for u in range(g):
                        jb = jb2 * 2 + u
                        nc.tensor.matmul(
                            ps[:, u, :],
                            kT[:, jb * 128 : (jb + 1) * 128],
                            qT[:, i2 * 512 : (i2 + 1) * 512],
                            start=True,
                            stop=True,
                        )
                    warm()
                    est = expst_pool.tile([128, 2, 512], dt_pv, tag="expst")
                    nc.scalar.activation(
                        est[:, :g, :],
                        ps[:, :g, :],
                        mybir.ActivationFunctionType.Exp,
                        scale=SCALE,
                    )
                    for u in range(g):
                        jb = jb2 * 2 + u
                        nc.tensor.matmul(
                            pv[:],
                            vv[:, jb, :],
                            est[:, u, :],
                            start=(jb == 0),
                            stop=(jb == NB - 1),
                        )
                outu = small.tile([HS, 512], dt_proj, tag="outu")
                nc.vector.tensor_copy(outu[:], pv[:])
                for c in range(KC):
                    ps = psum.tile([128, 2, 512], f32, tag="mm")
                    nc.tensor.matmul(
                        ps[:, 0, :],
                        wproj[:, c * 128 : (c + 1) * 128],
                        outu[:],
                        start=True,
                        stop=True,
                    )
                    pj = work.tile([128, 512], f32, tag="pj")
                    nc.vector.tensor_copy(pj[:], ps[:, 0, :])
                    nc.sync.dma_start(
                        d_proju[
                            c * 128 : (c + 1) * 128, i2 * 512 : (i2 + 1) * 512
                        ],
                        pj[:],
                    )

    nc.compile()
    return nc


def _get_kernel(n_ctx: int, precision: str):
    key = (n_ctx, precision)
    if key not in _COMPILED:
        _COMPILED[key] = _gen_kernel(n_ctx, precision)
    return _COMPILED[key]


def run_cores(x, Wqkv, Wproj, n_ctx=N, precision=PRECISION, trace=False):
    """Run the 8-core kernel; returns (list of per-core result dicts, perf)."""
    from concourse.bass_utils import run_bass_kernel_spmd

    nc = _get_kernel(n_ctx, precision)
    cfg = _CONFIGS[precision]
    np_qkv, np_proj = _np_dt(cfg[0]), _np_dt(cfg[3])

    xT = np.ascontiguousarray(x.reshape(n_ctx, DM).T).astype(np_qkv)
    in_maps = []
    for h in range(NH):
        sl = slice(h * HS, (h + 1) * HS)
        wq = Wqkv[0 * DM + h * HS : 0 * DM + (h + 1) * HS]  # [64, 512]
        wk = Wqkv[1 * DM + h * HS : 1 * DM + (h + 1) * HS]
        wv = Wqkv[2 * DM + h * HS : 2 * DM + (h + 1) * HS]
        wqkvT = np.ascontiguousarray(
            np.concatenate([wq, wk, wv], axis=0).T
        ).astype(np_qkv)  # [512, 192]
        wprojT = np.ascontiguousarray(Wproj[:, sl].T).astype(np_proj)
        in_maps.append({"xT": xT, "wqkvT": wqkvT, "wprojT": wprojT})

    res = run_bass_kernel_spmd(
        nc, in_maps, core_ids=list(range(NH)), trace=trace
    )
    return res.results, res


def kernel(x, Wqkv, Wproj, bproj):
    x = np.asarray(x, dtype=np.float32)
    Wqkv = np.asarray(Wqkv, dtype=np.float32)
    Wproj = np.asarray(Wproj, dtype=np.float32)
    bproj = np.asarray(bproj, dtype=np.float32)

    results, _ = run_cores(x, Wqkv, Wproj)

    att = np.empty((B, NH, N, N), dtype=np.float32)
    out = np.zeros((N, DM), dtype=np.float32)
    for h in range(NH):
        r = results[h]
        att[0, h] = r["att"]
        rs = r["rowsum"].T.reshape(N).astype(np.float64)  # rowsum for query i
        out += (r["proju"] / rs[None, :]).T.astype(np.float32)
    out += bproj[None, :]
    return out.reshape(B, N, DM).astype(np.float32), att


# revision 11
# speedup vs baseline: 3.3620x; 1.2133x over previous
"""Multi-head attention (B=1, N=4096, NH=8, HS=64, DM=512) on 8 Trainium2
NeuronCores, head-parallel: core h computes head h end-to-end.

Per-core Bass/Tile kernel (head h):
  inputs:  xT      [512, 4096]  x transposed (shared across cores)
           wqkvT   [512, 192]   [Wq_h^T | Wk_h^T | Wv_h^T]
           wprojT  [64, 512]    Wproj[:, h*64:(h+1)*64]^T
  outputs: att     [4096, 4096] softmax probabilities for head h (normalized)
           proju   [512, 4096]  Wproj_h @ (expS^T @ v)  -- unnormalized out^T
           rowsum  [128, NB]    rowsum[p, ib] = sum_j exp(s[ib*128+p, j])

  pass 1 (att): S tiles [128 q-rows, 512 k-cols] on PE -> exp on ACT (with
    accum_out rowsums) -> normalize on DVE -> DMA att rows out.
  pass 2 (out): S^T tiles [128 k-rows, 512 q-cols] on PE -> exp on ACT ->
    PV matmuls accumulating expS^T-weighted v -> projection matmuls -> DMA.

Host: out[i, :] = sum_h proju_h[:, i] / rowsum_h[i] + bproj; att stacked.

Per-stage matmul dtypes are configurable: float32r (TF32-like ~11-bit
mantissa, 2 cyc/row) / bfloat16 (1 cyc/row, keeps the PE HAM clock-gate
warm) / float32 (exact, 4 cyc/row). fp32-family matmuls do not register
as PE activity in the HAM monitor, so optional tiny bf16 "warmer"
matmuls keep the clock at 2.4 GHz.
"""

import numpy as np

B, N, NH, HS = 1, 4096, 8, 64
DM = NH * HS  # 512
SCALE = HS ** (-0.5)

PRECISION = "f32r"  # "f32r" | "bf16" | "mixed" | "f32"

_COMPILED = {}

# stage dtype table: (qkv, s, pv, proj, warmers)
_CONFIGS = {
    "f32r": ("float32r", "float32r", "float32r", "float32r", False),
    "bf16": ("bfloat16", "bfloat16", "bfloat16", "bfloat16", False),
    "mixed": ("float32r", "float32r", "bfloat16", "float32r", False),
    "f32": ("float32", "float32", "float32", "float32", True),
    # bisect configs for the bf16@4096 crash
    "s16": ("float32r", "bfloat16", "bfloat16", "float32r", False),
    "qkv16": ("bfloat16", "float32r", "float32r", "float32r", False),
    # v2: all-bf16 with PE array packing (row-packed S/S^T, col-packed PV)
    "bf16p": ("bfloat16", "bfloat16", "bfloat16", "bfloat16", False),
}


def _np_dt(name):
    import ml_dtypes

    return {
        "float32r": np.float32,
        "float32": np.float32,
        "bfloat16": ml_dtypes.bfloat16,
    }[name]


def _gen_kernel_packed(n_ctx: int):
    """v3: all-bf16 with PE array packing.

    S/S^T are K=64 contractions -> 2x row tiling: tile T0 reads SBUF
    partitions 0-63 (qT/kT), tile T8 reads partitions 64-127 (qk2, a
    DMA-made copy of qT/kT), concurrent on disjoint row halves of the
    PE. PV has M=64 -> 2x col tiling: even/odd j-chunks accumulate into
    psum partitions 0-63 / 64-127; the projection matmul contracts K=128
    against row-duplicated Wproj^T, summing the halves for free.
    QKV q/k are col-packed (q -> psum 0-63, k -> psum 64-127, shared x
    stream). Pass 2 is software-pipelined: PV/proj of block i2-1 are
    emitted between the ACT-gated S^T pairs of block i2 to fill PE idle.
    """
    import concourse.bacc as bacc
    import concourse.tile as tile
    from concourse import mybir

    f32 = mybir.dt.float32
    bf16 = mybir.dt.bfloat16

    NB = n_ctx // 128
    NJ = n_ctx // 512
    NB2 = (NB + 1) // 2
    KC = DM // 128
    NJ4 = (NJ + 3) // 4  # pass-1 groups of 4 j-blocks (one N=2048 ACT op)

    nc = bacc.Bacc(None, target_bir_lowering=False)

    d_xt = nc.dram_tensor("xT", [DM, n_ctx], bf16, kind="ExternalInput")
    d_wqkv = nc.dram_tensor("wqkvT", [DM, 3 * HS], bf16, kind="ExternalInput")
    d_wproj = nc.dram_tensor("wprojT", [HS, DM], bf16, kind="ExternalInput")
    d_att = nc.dram_tensor("att", [n_ctx, n_ctx], f32, kind="ExternalOutput")
    d_proju = nc.dram_tensor("proju", [DM, n_ctx], f32, kind="ExternalOutput")
    d_rowsum = nc.dram_tensor("rowsum", [128, NB], f32, kind="ExternalOutput")

    with tile.TileContext(nc) as tc:
        with (
            tc.tile_pool(name="persist", bufs=1) as persist,
            tc.tile_pool(name="work", bufs=3) as work,
            tc.tile_pool(name="small", bufs=4) as small,
            tc.tile_pool(name="estp", bufs=2) as estp,
            tc.tile_pool(name="xpool", bufs=1) as xpool,
        ):
            # ---- load inputs ----
            xt = xpool.tile([128, KC, n_ctx], bf16)
            nc.sync.dma_start(xt[:], d_xt[:].rearrange("(c p) n -> p c n", p=128))
            wqkv = xpool.tile([128, KC, 3 * HS], bf16)
            nc.sync.dma_start(
                wqkv[:], d_wqkv[:].rearrange("(c p) w -> p c w", p=128)
            )
            wproj2 = persist.tile([128, DM], bf16)  # Wproj_h^T duplicated rows
            nc.sync.dma_start(wproj2[:HS, :], d_wproj[:])
            nc.sync.dma_start(wproj2[HS:, :], d_wproj[:])

            qT = persist.tile([HS, n_ctx], bf16, tag="qT")
            kT = persist.tile([HS, n_ctx], bf16, tag="kT")
            qk2 = persist.tile([128, 2, n_ctx], bf16, tag="qk2")  # rows 64-127
            vv = persist.tile([128, NB, HS], bf16, tag="v")
            rowsum_sb = persist.tile([128, NB], f32, tag="rowsum")

            # ======== scope 1: qkv + pass 1 (8 psum banks) ========
            with tc.tile_pool(name="psA", bufs=2, space="PSUM") as psA:
                # q/k col-packed: T0 -> q (psum 0-63), T1 -> k (psum 64-127)
                scope_qkv = nc.named_scope("qkv")
                scope_qkv.__enter__()
                for nb in range(NJ):
                    ps = psA.tile([128, 4, 512], f32, tag="mm")
                    for c in range(KC):
                        nc.tensor.matmul(
                            ps[:HS, 0, :],
                            wqkv[:, c, 0:HS],
                            xt[:, c, nb * 512 : (nb + 1) * 512],
                            start=(c == 0),
                            stop=(c == KC - 1),
                            tile_position=(0, 0),
                        )
                        nc.tensor.matmul(
                            ps[HS:, 0, :],
                            wqkv[:, c, HS : 2 * HS],
                            xt[:, c, nb * 512 : (nb + 1) * 512],
                            start=(c == 0),
                            stop=(c == KC - 1),
                            tile_position=(0, 64),
                        )
                    nc.vector.tensor_copy(
                        qT[:, nb * 512 : (nb + 1) * 512], ps[:HS, 0, :]
                    )
                    nc.vector.tensor_copy(
                        qk2[HS:, 1, nb * 512 : (nb + 1) * 512], ps[HS:, 0, :]
                    )
                for jb in range(NB):
                    ps = psA.tile([128, 4, 512], f32, tag="mm")
                    for c in range(KC):
                        nc.tensor.matmul(
                            ps[:, 0, :HS],
                            xt[:, c, jb * 128 : (jb + 1) * 128],
                            wqkv[:, c, 2 * HS : 3 * HS],
                            start=(c == 0),
                            stop=(c == KC - 1),
                        )
                    nc.vector.tensor_copy(vv[:, jb, :], ps[:, 0, :HS])
                # distribute q/k copies for the second row half
                nc.sync.dma_start(kT[:, :], qk2[HS:, 1, :])
                nc.sync.dma_start(qk2[HS:, 0, :], qT[:, :])
                scope_qkv.__exit__(None, None, None)

                # ---- pass 1: S -> exp(accum) -> normalize -> att ----
                scope_p1 = nc.named_scope("p1")
                scope_p1.__enter__()
                for ib in range(NB):
                    ibs = slice(ib * 128, (ib + 1) * 128)
                    exps = work.tile([128, n_ctx], f32, tag="exps")
                    rsp = small.tile([128, NJ4], f32, tag="rsp")
                    for nb4 in range(NJ4):
                        g = min(4, NJ - nb4 * 4)  # j-blocks in this group
                        ps = psA.tile([128, 4, 512], f32, tag="mm")
                        for u in range(0, g, 2):
                            nb = nb4 * 4 + u
                            nc.tensor.matmul(
                                ps[:, u, :],
                                qT[:, ibs],
                                kT[:, nb * 512 : (nb + 1) * 512],
                                start=True,
                                stop=True,
                                tile_position=(0, 0),
                            )
                            if u + 1 < g:
                                nc.tensor.matmul(
                                    ps[:, u + 1, :],
                                    qk2[HS:, 0, ibs],
                                    qk2[
                                        HS:,
                                        1,
                                        (nb + 1) * 512 : (nb + 2) * 512,
                                    ],
                                    start=True,
                                    stop=True,
                                    tile_position=(64, 0),
                                )
                        nc.scalar.activation(
                            exps[:, nb4 * 2048 : nb4 * 2048 + g * 512],
                            ps[:, :g, :],
                            mybir.ActivationFunctionType.Exp,
                            scale=SCALE,
                            accum_out=rsp[:, nb4 : nb4 + 1],
                        )
                    rs = small.tile([128, 1], f32, tag="rs")
                    nc.vector.reduce_sum(
                        rs[:], rsp[:], axis=mybir.AxisListType.X
                    )
                    nc.vector.tensor_copy(rowsum_sb[:, ib : ib + 1], rs[:])
                    recip = small.tile([128, 1], f32, tag="recip")
                    nc.vector.reciprocal(recip[:], rs[:])
                    nc.vector.tensor_scalar_mul(exps[:], exps[:], recip[:])
                    nc.sync.dma_start(d_att[ibs, :], exps[:])
                nc.sync.dma_start(d_rowsum[:], rowsum_sb[:])
                scope_p1.__exit__(None, None, None)

            # ======== scope 2: pass 2 (6 + 2 psum banks) ========
            with (
                tc.tile_pool(name="psB", bufs=3, space="PSUM") as psB,
                tc.tile_pool(name="pvp", bufs=2, space="PSUM") as pvp,
            ):

                def st_pair(est, i2, jb2):
                    """row-packed S^T pair (jb = 2*jb2, 2*jb2+1) + exp."""
                    i2s = slice(i2 * 512, (i2 + 1) * 512)
                    g = min(2, NB - jb2 * 2)
                    ps = psB.tile([128, 2, 512], f32, tag="mm")
                    nc.tensor.matmul(
                        ps[:, 0, :],
                        kT[:, jb2 * 256 : jb2 * 256 + 128],
                        qT[:, i2s],
                        start=True,
                        stop=True,
                        tile_position=(0, 0),
                    )
                    if g == 2:
                        nc.tensor.matmul(
                            ps[:, 1, :],
                            qk2[HS:, 1, jb2 * 256 + 128 : jb2 * 256 + 256],
                            qk2[HS:, 0, i2s],
                            start=True,
                            stop=True,
                            tile_position=(64, 0),
                        )
                    nc.scalar.activation(
                        est[:, jb2 * 2 : jb2 * 2 + g, :],
                        ps[:, :g, :],
                        mybir.ActivationFunctionType.Exp,
                        scale=SCALE,
                    )

                def pv_pair(est, pv, jb2):
                    """col-packed PV pair accumulating into pv halves."""
                    g = min(2, NB - jb2 * 2)
                    nc.tensor.matmul(
                        pv[:HS, :],
                        vv[:, jb2 * 2, :],
                        est[:, jb2 * 2, :],
                        start=(jb2 == 0),
                        stop=(jb2 == NB2 - 1),
                        tile_position=(0, 0),
                        skip_group_check=True,
                    )
                    if g == 2:
                        nc.tensor.matmul(
                            pv[HS:, :],
                            vv[:, jb2 * 2 + 1, :],
                            est[:, jb2 * 2 + 1, :],
                            start=(jb2 == 0),
                            stop=(jb2 == NB2 - 1),
                            tile_position=(0, 64),
                            skip_group_check=True,
                        )

                def proj_out(pv, i2):
                    i2s = slice(i2 * 512, (i2 + 1) * 512)
                    outu = small.tile([128, 512], bf16, tag="outu")
                    nc.vector.tensor_copy(outu[:], pv[:])
                    for c in range(KC):
                        ps = psB.tile([128, 2, 512], f32, tag="mm")
                        nc.tensor.matmul(
                            ps[:, 0, :],
                            wproj2[:, c * 128 : (c + 1) * 128],
                            outu[:],
                            start=True,
                            stop=True,
                        )
                        pj = work.tile([128, 512], f32, tag="pj")
                        nc.vector.tensor_copy(pj[:], ps[:, 0, :])
                        nc.sync.dma_start(
                            d_proju[c * 128 : (c + 1) * 128, i2s], pj[:]
                        )

                # software pipeline over i2: S^T/exp of i2 interleaved (in
                # batches of 2 pairs) with PV of i2-1; proj of i2-1 at end.
                scope_p2 = nc.named_scope("p2")
                scope_p2.__enter__()
                ests = {}
                pvs = {}
                ests[0] = estp.tile([128, NB, 512], bf16, tag="est", name="est0")
                for jb2 in range(NB2):
                    st_pair(ests[0], 0, jb2)
                for i2 in range(1, NJ + 1):
                    pvs[i2 - 1] = pvp.tile([128, 512], f32, tag="pv",
                                           name=f"pv{i2 - 1}")
                    if i2 < NJ:
                        ests[i2] = estp.tile([128, NB, 512], bf16, tag="est",
                                             name=f"est{i2}")
                    for jb2 in range(0, NB2, 2):
                        if i2 < NJ:
                            for k in range(jb2, min(jb2 + 2, NB2)):
                                st_pair(ests[i2], i2, k)
                        for k in range(jb2, min(jb2 + 2, NB2)):
                            pv_pair(ests[i2 - 1], pvs[i2 - 1], k)
                    proj_out(pvs[i2 - 1], i2 - 1)
                scope_p2.__exit__(None, None, None)

    nc.compile()
    return nc


def _gen_kernel(n_ctx: int, precision: str):
    import concourse.bacc as bacc
    import concourse.tile as tile
    from concourse import mybir

    if precision == "bf16p":
        return _gen_kernel_packed(n_ctx)

    f32 = mybir.dt.float32
    bf16 = mybir.dt.bfloat16
    cfg = _CONFIGS[precision]
    dt_qkv, dt_s, dt_pv, dt_proj = (getattr(mybir.dt, n) for n in cfg[:4])
    warmers = cfg[4]

    NB = n_ctx // 128   # query/key 128-row blocks
    NJ = n_ctx // 512   # 512-wide column blocks
    NJ2 = (NJ + 1) // 2
    NB2 = (NB + 1) // 2
    KC = DM // 128      # 4 contraction chunks for qkv projections

    nc = bacc.Bacc(None, target_bir_lowering=False)

    d_xt = nc.dram_tensor("xT", [DM, n_ctx], dt_qkv, kind="ExternalInput")
    d_wqkv = nc.dram_tensor("wqkvT", [DM, 3 * HS], dt_qkv, kind="ExternalInput")
    d_wproj = nc.dram_tensor("wprojT", [HS, DM], dt_proj, kind="ExternalInput")
    d_att = nc.dram_tensor("att", [n_ctx, n_ctx], f32, kind="ExternalOutput")
    d_proju = nc.dram_tensor("proju", [DM, n_ctx], f32, kind="ExternalOutput")
    d_rowsum = nc.dram_tensor("rowsum", [128, NB], f32, kind="ExternalOutput")

    with tile.TileContext(nc) as tc:
        with (
            tc.tile_pool(name="persist", bufs=1) as persist,
            tc.tile_pool(name="work", bufs=3) as work,
            tc.tile_pool(name="small", bufs=4) as small,
            tc.tile_pool(name="expst", bufs=3) as expst_pool,
            tc.tile_pool(name="psum", bufs=2, space="PSUM") as psum,
            tc.tile_pool(name="psum_pv", bufs=2, space="PSUM") as psum_pv,
        ):
            # ---- HAM warmers: tiny bf16 matmuls that count as PE activity
            # (fp32-mode matmuls don't), keeping the clock gate at 2.4 GHz.
            if warmers:
                wsrc = persist.tile([1, 64], bf16, tag="wsrc")
                nc.vector.memset(wsrc[:], 0.0)
                wps = psum_pv.tile([1, 64], f32, tag="warm")

                def warm():
                    nc.tensor.matmul(
                        wps[:], wsrc[:, :1], wsrc[:], start=True, stop=True
                    )
            else:

                def warm():
                    pass

            # ---- load inputs ----
            xt = persist.tile([128, KC, n_ctx], dt_qkv)  # xT[c*128+p, :]
            nc.sync.dma_start(
                xt[:], d_xt[:].rearrange("(c p) n -> p c n", p=128)
            )
            wqkv = persist.tile([128, KC, 3 * HS], dt_qkv)
            nc.sync.dma_start(
                wqkv[:], d_wqkv[:].rearrange("(c p) w -> p c w", p=128)
            )
            wproj = persist.tile([HS, DM], dt_proj)
            nc.sync.dma_start(wproj[:], d_wproj[:])

            # ---- qkv projections ----
            # qT/kT [64, n_ctx] = W^T.T @ xT ; v [128, jb, 64] plain
            qT = persist.tile([HS, n_ctx], dt_s, tag="qT")
            kT = persist.tile([HS, n_ctx], dt_s, tag="kT")
            vv = persist.tile([128, NB, HS], dt_pv, tag="v")
            for nb in range(NJ):
                for t, w0 in ((qT, 0), (kT, HS)):
                    ps = psum.tile([128, 2, 512], f32, tag="mm")
                    for c in range(KC):
                        nc.tensor.matmul(
                            ps[:HS, 0, :],
                            wqkv[:, c, w0 : w0 + HS],
                            xt[:, c, nb * 512 : (nb + 1) * 512],
                            start=(c == 0),
                            stop=(c == KC - 1),
                        )
                    warm()
                    nc.vector.tensor_copy(
                        t[:, nb * 512 : (nb + 1) * 512], ps[:HS, 0, :]
                    )
            for jb in range(NB):
                ps = psum.tile([128, 2, 512], f32, tag="mm")
                for c in range(KC):
                    nc.tensor.matmul(
                        ps[:, 0, :HS],
                        xt[:, c, jb * 128 : (jb + 1) * 128],
                        wqkv[:, c, 2 * HS : 3 * HS],
                        start=(c == 0),
                        stop=(c == KC - 1),
                    )
                if jb % 2 == 0:
                    warm()
                nc.vector.tensor_copy(vv[:, jb, :], ps[:, 0, :HS])

            # ---- pass 1: S -> exp -> normalize -> att ----
            # j-blocks in pairs: two matmuls fill the two banks of one psum
            # tile, one ACT op exps over both (N=1024 amortizes overhead).
            rowsum_sb = persist.tile([128, NB], f32, tag="rowsum")
            for ib in range(NB):
                exps = work.tile([128, n_ctx], f32, tag="exps")
                rsp = small.tile([128, NJ2], f32, tag="rsp")
                for nb2 in range(NJ2):
                    g = min(2, NJ - nb2 * 2)
                    ps = psum.tile([128, 2, 512], f32, tag="mm")
                    for u in range(g):
                        nb = nb2 * 2 + u
                        nc.tensor.matmul(
                            ps[:, u, :],
                            qT[:, ib * 128 : (ib + 1) * 128],
                            kT[:, nb * 512 : (nb + 1) * 512],
                            start=True,
                            stop=True,
                        )
                    warm()
                    nc.scalar.activation(
                        exps[:, nb2 * 1024 : nb2 * 1024 + g * 512],
                        ps[:, :g, :],
                        mybir.ActivationFunctionType.Exp,
                        scale=SCALE,
                        accum_out=rsp[:, nb2 : nb2 + 1],
                    )
                rs = small.tile([128, 1], f32, tag="rs")
                nc.vector.reduce_sum(rs[:], rsp[:], axis=mybir.AxisListType.X)
                nc.vector.tensor_copy(rowsum_sb[:, ib : ib + 1], rs[:])
                recip = small.tile([128, 1], f32, tag="recip")
                nc.vector.reciprocal(recip[:], rs[:])
                nc.vector.tensor_scalar_mul(exps[:], exps[:], recip[:])
                nc.sync.dma_start(d_att[ib * 128 : (ib + 1) * 128, :], exps[:])
            nc.sync.dma_start(d_rowsum[:], rowsum_sb[:])

            # ---- pass 2: S^T -> exp -> PV -> projection ----
            for i2 in range(NJ):
                pv = psum_pv.tile([HS, 512], f32, tag="pv")
                for jb2 in range(NB2):
                    g = min(2, NB - jb2 * 2)
                    ps = psum.tile([128, 2, 512], f32, tag="mm")
                    for u in range(g):
                        jb = jb2 * 2 + u
                        nc.tensor.matmul(
                            ps[:, u, :],
                            kT[:, jb * 128 : (jb + 1) * 128],
                            qT[:, i2 * 512 : (i2 + 1) * 512],
                            start=True,
                            stop=True,
                        )
                    warm()
                    est = expst_pool.tile([128, 2, 512], dt_pv, tag="expst")
                    nc.scalar.activation(
                        est[:, :g, :],
                        ps[:, :g, :],
                        mybir.ActivationFunctionType.Exp,
                        scale=SCALE,
                    )
                    for u in range(g):
                        jb = jb2 * 2 + u
                        nc.tensor.matmul(
                            pv[:],
                            vv[:, jb, :],
                            est[:, u, :],
                            start=(jb == 0),
                            stop=(jb == NB - 1),
                        )
                outu = small.tile([HS, 512], dt_proj, tag="outu")
                nc.vector.tensor_copy(outu[:], pv[:])
                for c in range(KC):
                    ps = psum.tile([128, 2, 512], f32, tag="mm")
                    nc.tensor.matmul(
                        ps[:, 0, :],
                        wproj[:, c * 128 : (c + 1) * 128],
                        outu[:],
                        start=True,
                        stop=True,
                    )
                    pj = work.tile([128, 512], f32, tag="pj")
                    nc.vector.tensor_copy(pj[:], ps[:, 0, :])
                    nc.sync.dma_start(
                        d_proju[
                            c * 128 : (c + 1) * 128, i2 * 512 : (i2 + 1) * 512
                        ],
                        pj[:],
                    )

    nc.compile()
    return nc


def _get_kernel(n_ctx: int, precision: str):
    key = (n_ctx, precision)
    if key not in _COMPILED:
        _COMPILED[key] = _gen_kernel(n_ctx, precision)
    return _COMPILED[key]


def run_cores(x, Wqkv, Wproj, n_ctx=N, precision=PRECISION, trace=False):
    """Run the 8-core kernel; returns (list of per-core result dicts, perf)."""
    from concourse.bass_utils import run_bass_kernel_spmd

    nc = _get_kernel(n_ctx, precision)
    cfg = _CONFIGS[precision]
    np_qkv, np_proj = _np_dt(cfg[0]), _np_dt(cfg[3])

    xT = np.ascontiguousarray(x.reshape(n_ctx, DM).T).astype(np_qkv)
    in_maps = []
    for h in range(NH):
        sl = slice(h * HS, (h + 1) * HS)
        wq = Wqkv[0 * DM + h * HS : 0 * DM + (h + 1) * HS]  # [64, 512]
        wk = Wqkv[1 * DM + h * HS : 1 * DM + (h + 1) * HS]
        wv = Wqkv[2 * DM + h * HS : 2 * DM + (h + 1) * HS]
        wqkvT = np.ascontiguousarray(
            np.concatenate([wq, wk, wv], axis=0).T
        ).astype(np_qkv)  # [512, 192]
        wprojT = np.ascontiguousarray(Wproj[:, sl].T).astype(np_proj)
        in_maps.append({"xT": xT, "wqkvT": wqkvT, "wprojT": wprojT})

    res = run_bass_kernel_spmd(
        nc, in_maps, core_ids=list(range(NH)), trace=trace
    )
    return res.results, res


def kernel(x, Wqkv, Wproj, bproj):
    x = np.asarray(x, dtype=np.float32)
    Wqkv = np.asarray(Wqkv, dtype=np.float32)
    Wproj = np.asarray(Wproj, dtype=np.float32)
    bproj = np.asarray(bproj, dtype=np.float32)

    results, _ = run_cores(x, Wqkv, Wproj)

    att = np.empty((B, NH, N, N), dtype=np.float32)
    out = np.zeros((N, DM), dtype=np.float32)
    for h in range(NH):
        r = results[h]
        att[0, h] = r["att"]
        rs = r["rowsum"].T.reshape(N).astype(np.float64)  # rowsum for query i
        out += (r["proju"] / rs[None, :]).T.astype(np.float32)
    out += bproj[None, :]
    return out.reshape(B, N, DM).astype(np.float32), att
